# revision 1
# baseline (speedup 1.0000x reference)
"""nn_Attention Trainium2 Bass kernel — data-parallel over batch on 8 NeuronCores.

Per core (one batch element): full attention
  qh = q@Wq + bq; kh = k@Wk + bk; vh = kh@Wv + bv
  scores = qh@kh.T  (+ mask -> -10000); probs = softmax(scores)
  out = (probs @ vh) @ Wo + bo

Device-side algebra (per core):
  khT[h,l]   = Wk.T-tiles @ kT          (+bk per-partition)      [f32r]
  vh0[l,v]   = khT.T-tiles @ Wv         (no bias; bv folded into bo2)  [bf16]
  WqKhT[d,k] = WqT.T-tiles @ khT        ( = Wq @ khT )           [f32r]
  bqRow[k]   = bq.T @ khT
  biasRep    = broadcast(maskBias + bqRow) over 128 partitions
  per q-tile: scores = qT.T-tiles @ WqKhT  [+ biasRep]
              softmax rowwise (max, exp with accum-sum, reciprocal)
              probsT via PE transposes (bf16)
  outUT[h,q] = vh0-tiles.T @ probsT                              [f32r out]
  finalT[v,q]= Wo-tiles.T @ outUT       (+bo2 per-partition)     [f32]
Host: out[b] = finalT.T ;  bo2 = bv@Wo + bo  (exact algebra: probs rows sum to 1)
"""
import numpy as np

import concourse.bass as bass
import concourse.mybir as mybir
from concourse import bacc, tile
from concourse.bass_utils import run_bass_kernel_spmd
from concourse.masks import make_identity

B, L, D, H = 8, 2048, 1024, 1024
P = 128
F32 = mybir.dt.float32
F32R = mybir.dt.float32r
BF16 = mybir.dt.bfloat16
AF = mybir.ActivationFunctionType
AX = mybir.AxisListType

QBLK = 512          # q columns per outer B-phase block
NQB = L // QBLK     # 8
LT = L // P         # 16 l/k tiles
DT = D // P         # 8 d tiles
HT = H // P         # 8 h tiles


def build_nc():
    nc = bacc.Bacc("TRN2", target_bir_lowering=False, debug=False, num_devices=8)
    qt_d = nc.dram_tensor("qt", [D, L], F32R, kind="ExternalInput").ap()
    kt_d = nc.dram_tensor("kt", [D, L], F32R, kind="ExternalInput").ap()
    wk_d = nc.dram_tensor("wk", [D, H], F32R, kind="ExternalInput").ap()
    wv_d = nc.dram_tensor("wv", [H, H], BF16, kind="ExternalInput").ap()
    wqt_d = nc.dram_tensor("wqt", [H, D], F32R, kind="ExternalInput").ap()
    wo_d = nc.dram_tensor("wo", [H, D], BF16, kind="ExternalInput").ap()
    bk_d = nc.dram_tensor("bk", [H, 1], F32, kind="ExternalInput").ap()
    bq_d = nc.dram_tensor("bq", [H, 1], F32R, kind="ExternalInput").ap()
    bo2_d = nc.dram_tensor("bo2", [D, 1], F32, kind="ExternalInput").ap()
    maskb_d = nc.dram_tensor("maskb", [1, L], F32, kind="ExternalInput").ap()
    ones_d = nc.dram_tensor("ones", [1, P], F32R, kind="ExternalInput").ap()
    out_d = nc.dram_tensor("out", [D, L], F32, kind="ExternalOutput").ap()

    spill_r = [nc.dram_tensor(f"khT_r{i}", [H, 512], F32R).ap() for i in range(4)]
    spill_b = [nc.dram_tensor(f"khT_b{i}", [H, 512], BF16).ap() for i in range(4)]

    with tile.TileContext(nc) as tc:
        with tc.tile_pool(name="const", bufs=1) as cp, \
             tc.tile_pool(name="persistB", bufs=1) as pB:
            bk_t = cp.tile([P, HT], F32)
            bq_t = cp.tile([P, HT], F32R)
            bo2_t = cp.tile([P, DT], F32)
            for i in range(HT):
                nc.gpsimd.dma_start(out=bk_t[:, i:i + 1], in_=bk_d[i * P:(i + 1) * P, :])
                nc.gpsimd.dma_start(out=bq_t[:, i:i + 1], in_=bq_d[i * P:(i + 1) * P, :])
                nc.gpsimd.dma_start(out=bo2_t[:, i:i + 1], in_=bo2_d[i * P:(i + 1) * P, :])
            onesr_t = cp.tile([1, P], F32R)
            nc.gpsimd.dma_start(out=onesr_t, in_=ones_d)
            ident_f = cp.tile([P, P], F32)
            make_identity(nc, ident_f)
            identb_t = cp.tile([P, P], BF16)
            nc.vector.tensor_copy(identb_t, ident_f)
            biasrep_t = cp.tile([P, L], F32)

            vh_t = pB.tile([P, LT, H], BF16)
            wqkh = []
            for i in range(4):
                wq_i = pB.tile([P, DT, 512], F32R, tag=f"wqkh{i}")
                wqkh.append(wq_i)

            # ---------- A1: khT = Wk.T-tiles @ kT (+bk) -> spills (f32r + bf16)
            with tc.tile_pool(name="a1w", bufs=1) as wp, \
                 tc.tile_pool(name="a1s", bufs=2) as sp_, \
                 tc.tile_pool(name="a1ps", bufs=2, space="PSUM") as pp:
                wk_t = wp.tile([P, DT, H], F32R, tag="w")
                for lb in range(4):
                    ktb = sp_.tile([P, DT, 512], F32R, tag="ktb")
                    for d in range(DT):
                        if lb == 0:
                            nc.sync.dma_start(out=wk_t[:, d],
                                              in_=wk_d[d * P:(d + 1) * P, :])
                        nc.sync.dma_start(
                            out=ktb[:, d],
                            in_=kt_d[d * P:(d + 1) * P, lb * 512:(lb + 1) * 512])
                    for ht in range(HT):
                        ps = pp.tile([P, 512], F32, tag="ps")
                        for d in range(DT):
                            nc.tensor.matmul(ps, wk_t[:, d, ht * P:(ht + 1) * P],
                                             ktb[:, d], start=(d == 0), stop=(d == DT - 1))
                        st = sp_.tile([P, 512], F32R, tag="khst")
                        nc.scalar.activation(st, ps, AF.Identity, bias=bk_t[:, ht:ht + 1])
                        nc.sync.dma_start(out=spill_r[lb][ht * P:(ht + 1) * P, :], in_=st)
                        stb = sp_.tile([P, 512], BF16, tag="khstb")
                        nc.vector.tensor_copy(stb, st)
                        nc.sync.dma_start(out=spill_b[lb][ht * P:(ht + 1) * P, :], in_=stb)

            # ---------- A2: vh0 = khT.T-tiles @ Wv (all bf16)
            with tc.tile_pool(name="a2w", bufs=1) as wp, \
                 tc.tile_pool(name="a2s", bufs=2) as sp_, \
                 tc.tile_pool(name="a2ps", bufs=2, space="PSUM") as pp:
                wv_t = wp.tile([P, HT, H], BF16, tag="w")
                for i in range(HT):
                    nc.sync.dma_start(out=wv_t[:, i], in_=wv_d[i * P:(i + 1) * P, :])
                for lb in range(4):
                    khb = sp_.tile([P, HT, 512], BF16, tag="khb")
                    for h in range(HT):
                        nc.sync.dma_start(out=khb[:, h],
                                          in_=spill_b[lb][h * P:(h + 1) * P, :])
                    for lt in range(4):
                        l_idx = lb * 4 + lt
                        for vb in range(2):
                            ps = pp.tile([P, 512], F32, tag="ps")
                            for h in range(HT):
                                nc.tensor.matmul(
                                    ps, khb[:, h, lt * P:(lt + 1) * P],
                                    wv_t[:, h, vb * 512:(vb + 1) * 512],
                                    start=(h == 0), stop=(h == HT - 1))
                            nc.scalar.activation(
                                vh_t[:, l_idx, vb * 512:(vb + 1) * 512], ps, AF.Copy)

            # early prefetch of first q-block on the scalar HWDGE queue
            bq1_cm = tc.tile_pool(name="bq1", bufs=1, side="right")
            qp1 = bq1_cm.__enter__()
            first_q = qp1.tile([P, DT, QBLK], F32R, tag="qtb0")
            for d in range(DT):
                nc.scalar.dma_start(out=first_q[:, d], in_=qt_d[d * P:(d + 1) * P, 0:QBLK])

            # ---------- A3: WqKhT = WqT.T-tiles @ khT ; biasRep per kb
            with tc.tile_pool(name="a3w", bufs=1) as wp, \
                 tc.tile_pool(name="a3s", bufs=2) as sp_, \
                 tc.tile_pool(name="a3ps", bufs=2, space="PSUM") as pp:
                wqt_t = wp.tile([P, HT, D], F32R, tag="w")
                for i in range(HT):
                    nc.sync.dma_start(out=wqt_t[:, i], in_=wqt_d[i * P:(i + 1) * P, :])
                for kb in range(4):
                    khb = sp_.tile([P, HT, 512], F32R, tag="khb")
                    for h in range(HT):
                        nc.sync.dma_start(out=khb[:, h],
                                          in_=spill_r[kb][h * P:(h + 1) * P, :])
                    for dt in range(DT):
                        ps = pp.tile([P, 512], F32, tag="ps")
                        for h in range(HT):
                            nc.tensor.matmul(ps, wqt_t[:, h, dt * P:(dt + 1) * P],
                                             khb[:, h], start=(h == 0), stop=(h == HT - 1))
                        nc.scalar.activation(wqkh[kb][:, dt], ps, AF.Copy)
                    ps1 = pp.tile([1, 512], F32, tag="bqps")
                    for h in range(HT):
                        nc.tensor.matmul(ps1, bq_t[:, h:h + 1], khb[:, h],
                                         start=(h == 0), stop=(h == HT - 1))
                    brow = sp_.tile([1, 512], F32, tag="brow", bufs=1)
                    nc.scalar.activation(brow, ps1, AF.Copy)
                    mrow = sp_.tile([1, 512], F32, tag="mrow", bufs=1)
                    nc.sync.dma_start(out=mrow, in_=maskb_d[:, kb * 512:(kb + 1) * 512])
                    nc.vector.tensor_add(brow, brow, mrow)
                    browr = sp_.tile([1, 512], F32R, tag="browr", bufs=1)
                    nc.vector.tensor_copy(browr, brow)
                    ps2 = pp.tile([P, 512], F32, tag="brps")
                    nc.tensor.matmul(ps2, onesr_t, browr, start=True, stop=True)
                    nc.scalar.activation(biasrep_t[:, kb * 512:(kb + 1) * 512], ps2, AF.Copy)

            # ---------- B: attention per q-block
            with tc.tile_pool(name="bw", bufs=1) as wp_b, \
                 tc.tile_pool(name="bq2", bufs=2) as qp, \
                 tc.tile_pool(name="bsm", bufs=2) as smp, \
                 tc.tile_pool(name="bpt", bufs=1) as ptp, \
                 tc.tile_pool(name="bou", bufs=1) as oup, \
                 tc.tile_pool(name="bst", bufs=1) as stp, \
                 tc.tile_pool(name="bps_s", bufs=1, space="PSUM") as pss, \
                 tc.tile_pool(name="bps_t", bufs=2, space="PSUM") as pst, \
                 tc.tile_pool(name="bps_m", bufs=2, space="PSUM") as psm:
                wo_t = wp_b.tile([P, HT, D], BF16, tag="w")
                for i in range(HT):
                    nc.scalar.dma_start(out=wo_t[:, i], in_=wo_d[i * P:(i + 1) * P, :])
                qtbs = {0: first_q}
                for qb in range(NQB):
                    qtb = qtbs.pop(qb)
                    probsT = ptp.tile([P, LT, QBLK], BF16, tag="probsT")

                    for qt in range(QBLK // P):
                        sps = pss.tile([P, L], F32, tag="scps")
                        for kb in range(4):
                            for d in range(DT):
                                nc.tensor.matmul(
                                    sps[:, kb * 512:(kb + 1) * 512],
                                    qtb[:, d, qt * P:(qt + 1) * P],
                                    wqkh[kb][:, d],
                                    start=(d == 0), stop=(d == DT - 1))
                        s_t = smp.tile([P, L], F32, tag="s", bufs=1)
                        nc.vector.tensor_add(s_t, sps, biasrep_t)
                        mx = smp.tile([P, 1], F32, tag="mx")
                        nc.vector.reduce_max(mx, s_t, axis=AX.X)
                        negmx = smp.tile([P, 1], F32, tag="negmx")
                        nc.vector.tensor_scalar_mul(negmx, mx, -1.0)
                        p_t = smp.tile([P, L], BF16, tag="pp", bufs=1)
                        sume = smp.tile([P, 1], F32, tag="sume")
                        nc.scalar.activation(p_t, s_t, AF.Exp, bias=negmx, scale=1.0,
                                             accum_out=sume)
                        recip = smp.tile([P, 1], F32, tag="recip")
                        nc.vector.reciprocal(recip, sume)
                        nc.vector.tensor_scalar_mul(p_t, p_t, recip)
                        for kt in range(LT):
                            tp = pst.tile([P, P], BF16, tag="tp")
                            nc.tensor.transpose(tp, p_t[:, kt * P:(kt + 1) * P],
                                                identb_t)
                            nc.scalar.activation(
                                probsT[:, kt, qt * P:(qt + 1) * P], tp, AF.Copy)

                    if qb == 0:
                        bq1_cm.__exit__(None, None, None)
                    if qb + 1 < NQB:
                        nxq = qp.tile([P, DT, QBLK], F32R, tag="qtb")
                        for d in range(DT):
                            nc.scalar.dma_start(
                                out=nxq[:, d],
                                in_=qt_d[d * P:(d + 1) * P,
                                         (qb + 1) * QBLK:(qb + 2) * QBLK])
                        qtbs[qb + 1] = nxq

                    outut = oup.tile([P, HT, QBLK], BF16, tag="outut")
                    for ht in range(HT):
                        ps = psm.tile([P, QBLK], F32, tag="mmps")
                        for kt in range(LT):
                            nc.tensor.matmul(ps, vh_t[:, kt, ht * P:(ht + 1) * P],
                                             probsT[:, kt], start=(kt == 0),
                                             stop=(kt == LT - 1))
                        nc.scalar.activation(outut[:, ht], ps, AF.Copy)

                    for vt in range(DT):
                        ps = psm.tile([P, QBLK], F32, tag="mmps")
                        for h in range(HT):
                            nc.tensor.matmul(ps, wo_t[:, h, vt * P:(vt + 1) * P],
                                             outut[:, h], start=(h == 0),
                                             stop=(h == HT - 1))
                        ot = stp.tile([P, QBLK], F32, tag="ot")
                        nc.scalar.activation(ot, ps, AF.Identity, bias=bo2_t[:, vt:vt + 1])
                        nc.sync.dma_start(
                            out=out_d[vt * P:(vt + 1) * P, qb * QBLK:(qb + 1) * QBLK],
                            in_=ot)
    nc.compile()
    return nc


_NC_CACHE = None


def _get_nc():
    global _NC_CACHE
    if _NC_CACHE is None:
        _NC_CACHE = build_nc()
    return _NC_CACHE


def kernel(q, k, mask, Wq, bq, Wk, bk, Wv, bv, Wo, bo):
    q = np.asarray(q, np.float32)
    k = np.asarray(k, np.float32)
    mask = np.asarray(mask)
    Wq = np.asarray(Wq, np.float32)
    Wk = np.asarray(Wk, np.float32)
    Wv = np.asarray(Wv, np.float32)
    Wo = np.asarray(Wo, np.float32)
    bq_ = np.asarray(bq, np.float32)
    bk_ = np.asarray(bk, np.float32)
    bv_ = np.asarray(bv, np.float32)
    bo_ = np.asarray(bo, np.float32)

    nc = _get_nc()
    import ml_dtypes
    wqt = np.ascontiguousarray(Wq.T)
    wo_bf = Wo.astype(ml_dtypes.bfloat16)
    wv_bf = Wv.astype(ml_dtypes.bfloat16)
    bo2 = (bv_.astype(np.float64) @ Wo.astype(np.float64) + bo_).astype(np.float32)
    ones = np.ones((1, P), np.float32)
    in_maps = []
    for b in range(B):
        maskb = ((mask[b].astype(np.float32) - 1.0) * 10000.0).astype(np.float32)
        in_maps.append({
            "qt": np.ascontiguousarray(q[b].T),
            "kt": np.ascontiguousarray(k[b].T),
            "wk": Wk, "wv": wv_bf, "wqt": wqt, "wo": wo_bf,
            "bk": bk_.reshape(H, 1), "bq": bq_.reshape(H, 1),
            "bo2": bo2.reshape(D, 1),
            "maskb": maskb.reshape(1, L),
            "ones": ones,
        })
    res = run_bass_kernel_spmd(nc, in_maps, core_ids=list(range(B)))
    out = np.stack([np.ascontiguousarray(res.results[b]["out"].T) for b in range(B)])
    return out.astype(np.float32)

